# revision 58
# baseline (speedup 1.0000x reference)
"""BlockMaskGenerator Trainium2 kernel.

Reference semantics: for each of 256 batch items, 4 random rectangular
blocks are placed on a 128x128 grid; target_mask = union of the blocks
(flattened to 16384), context_mask = complement.

Strategy:
  * Host: geometry via the same eager jax ops as the reference (the
    neuron backend's float->int cast rounds, numpy would truncate);
    1024 elements of scalar math - negligible.
  * Data-parallel over 8 NeuronCores: core d handles batches [32d, 32d+32)
    = blocks [128d, 128d+128).
  * Device per core. Output cell j = 128*b + c; interval indicator
    [L <= j < R] = [j >= L] - [j >= R] (difference of step functions).
    The output is computed in 2 halves of 16 batches = 64 blocks each,
    so the difference is absorbed into the K=128 matmul contraction:
    partitions 0-63 carry +row_q * [j >= L_q], partitions 64-127 carry
    -row_q * [j >= R_q] (sign flip encoded by swapping row thresholds
    host-side; all device ops are plain is_ge):
      - J[p, j'] = j' for j' in [0, 2048): Pool iota, 2 chunks in
        separate tiles (Tile deps are per-tile); both output halves
        reuse J with thresholds shifted by -2048
      - rhs[p, j'] = [j' >= colthr_h[p]]  (4 is_ge chunks on DVE, 4x)
      - lhsT_h[p, r] = [r >= A_h[p]] - [r >= B_h[p]]  (tiny, row iota
        rides in the geom input)
      - psum_g = lhsT_h.T @ rhs slice = # blocks covering cell (exact
        small ints >= 0)
      - target bytes = Copy(psum) -> uint8 counts on ScalarE (host maps
        nonzero -> True; no Sign act-table load)
      - context = (count == 0) on DVE (GpSimd ALU measured ~8us/op)
      - per-half output DMAs [128(r), 16(b), 128(c)] uint8; host
        transposes during unshard.
  * The framework's init all-engine barrier (~4us of cold drains) guards
    only unused const-tile memsets; it and the duplicated tail barriers
    are patched to cheaper forms at build time.
"""

import os

import numpy as np

NUM_TARGET_BLOCKS = 4
SCALE_MIN = 0.15
SCALE_MAX = 0.2
ASPECT_RATIO = 0.75
BATCH = 256
HEIGHT = 128
WIDTH = 128
SEQ_LEN = HEIGHT * WIDTH
N_CORES = 8
B_PER_CORE = BATCH // N_CORES                # 32
P_PER_CORE = B_PER_CORE * NUM_TARGET_BLOCKS  # 128 blocks per core
NFREE = B_PER_CORE * WIDTH                   # 4096
JW = NFREE // 2                              # 2048

_GRAPH = None
last_exec_time_ns = None
last_results = None


def _host_geometry(scales_u, rand_top, rand_left, height=HEIGHT, width=WIDTH):
    """Geometry computed with the SAME eager jax ops as the reference.

    The reference runs eagerly on this environment's jax backend (neuron),
    whose float->int cast rounds instead of truncating; replicating the
    exact op sequence on the same backend gives bit-identical geometry
    (verified: 0/4.2M mask mismatches, vs 1.3% for a numpy mirror).
    """
    import jax.numpy as jnp

    scales = SCALE_MIN + scales_u * (SCALE_MAX - SCALE_MIN)
    block_areas = (scales * height * width).astype(jnp.int32)
    block_hs = jnp.clip(
        jnp.sqrt(block_areas.astype(jnp.float32) / ASPECT_RATIO).astype(jnp.int32),
        1, height)
    block_ws = jnp.clip(
        (block_areas.astype(jnp.float32)
         / jnp.clip(block_hs, 1, None).astype(jnp.float32)).astype(jnp.int32),
        1, width)
    max_tops = jnp.clip(height - block_hs + 1, 1, None)
    max_lefts = jnp.clip(width - block_ws + 1, 1, None)
    tops = (rand_top * max_tops.astype(jnp.float32)).astype(jnp.int32)
    lefts = (rand_left * max_lefts.astype(jnp.float32)).astype(jnp.int32)
    return (np.asarray(tops), np.asarray(block_hs),
            np.asarray(lefts), np.asarray(block_ws))


def _build_graph():
    import concourse.bacc as bacc
    import concourse.bass as bass
    import concourse.tile as tile
    from concourse import mybir

    skip_init_barrier = os.environ.get("BMG_INIT_BARRIER", "skip") == "skip"
    if skip_init_barrier:
        # Bass.__init__'s all-engine barrier only guards const-ap memsets
        # (unused by this kernel: Copy keeps a float bias, compares use AP
        # scalars). Cold InstDrains cost ~3us per engine on HW.
        orig_barrier = bass.Bass.all_engine_barrier
        bass.Bass.all_engine_barrier = lambda self, **k: None

    try:
        # Bacc (not plain Bass): compile() runs generate_event_semaphores,
        # which splits multi-sem waits (TRN2 instructions have one
        # sync-wait slot; raw Tile output fails walrus codegen).
        nc = bacc.Bacc()
    finally:
        if skip_init_barrier:
            bass.Bass.all_engine_barrier = orig_barrier

    # geom columns: 0=rowA_h0 1=rowB_h0 2=rowA_h1 3=rowB_h1
    #               4=colthr_h0 5=colthr_h1 6=0 7=pad 8..135=row iota (r)
    geom = nc.declare_dram_parameter(
        "geom", [128, 8 + 256], mybir.dt.float32, isOutput=False)
    jconst = nc.declare_dram_parameter(
        "jconst", [1, JW], mybir.dt.int16, isOutput=False)
    tmask = nc.declare_dram_parameter(
        "tmask", [128, B_PER_CORE, WIDTH], mybir.dt.uint8, isOutput=True)
    cmask = nc.declare_dram_parameter(
        "cmask", [128, B_PER_CORE, WIDTH], mybir.dt.uint8, isOutput=True)

    A = mybir.AluOpType
    F = mybir.ActivationFunctionType

    if os.environ.get("BMG_TAIL_BARRIER", "sem") == "sem":
        # The tail's two all-engine barriers only order the semaphore
        # cleanup after the (kept) completion drain; sequencer-level
        # barriers suffice and skip another round of ~1us InstDrains.
        def _dab(self, tick_clock, wait_clock):
            drain_inst = self.nc.sync.drain()
            wait_clock.add_sem_waits(
                drain_inst.ins,
                tile.ScopedClock({None: tick_clock.global_clock}))
            hand = self.nc.alloc_semaphore("tail_handoff")
            drain_inst.then_inc(hand, 1)
            self.nc.gpsimd.wait_ge(hand, 1)
            popped = self.nc._tile_sem_poison_stack.pop()
            assert popped is self._sem_poison
            self.nc.clear_and_free_semaphores(
                list(self.sems.allocated().values()))
            self.nc.gpsimd.sem_clear(hand)
            # no trailing barrier: the clears are already the last
            # instructions on their engine queue; NRT waits for all
            # queues to drain at NEFF end regardless
        tc_cls = tile.TileContext
        orig_dab = tc_cls._drain_and_barrier
        tc_cls._drain_and_barrier = _dab
    else:
        orig_dab = None

    with tile.TileContext(nc) as tc:
        with (
            tc.tile_pool(name="sbuf", bufs=1) as pool,
            tc.tile_pool(name="tmp", bufs=2) as tmp,
            tc.tile_pool(name="psum", bufs=4, space="PSUM") as psum_pool,
        ):
            g = pool.tile([128, 8 + 256], mybir.dt.float32)
            nc.sync.dma_start(out=g, in_=geom[:, :])

            # J[p, j'] = j' in [0, 2048): partition-broadcast DMAs of a
            # host arange, 2 chunks in separate tiles (Tile deps are
            # per-tile). Pool iota is avoided - a running iota stalls
            # every concurrent DVE op (GpSimd/DVE SBUF port sharing).
            # Hybrid: chunk 0 via Pool iota (finishes before DVE has
            # inputs, so its port-conflict stall is free), chunk 1 via
            # partition-broadcast DMA (a second iota would stall DVE
            # right when work arrives).
            import concourse.bass as bass
            J0 = pool.tile([128, JW // 2], mybir.dt.int16, tag="J0")
            nc.gpsimd.iota(J0, pattern=[[1, JW // 2]], base=0,
                           channel_multiplier=0)
            jhalf = jconst.ap()[:, JW // 2:]
            J1 = pool.tile([128, JW // 2], mybir.dt.int16, tag="J1")
            nc.sync.dma_start(out=J1, in_=bass.AP(
                tensor=jhalf.tensor, offset=jhalf.offset,
                ap=[[0, 128]] + jhalf.ap[1:]))
            Jc = [J0, J1]

            # lhsT per half rides in the geom DMA as f32 (+1/0/-1,
            # host-computed); one convert replaces six compare/sub ops
            # on the pre-matmul critical path
            lhs = []
            for h in range(2):
                lh = pool.tile([128, 128], mybir.dt.bfloat16, tag=f"lh{h}")
                nc.vector.tensor_copy(
                    out=lh, in_=g[:, 8 + 128 * h: 136 + 128 * h])
                lhs.append(lh)

            rhs = pool.tile([128, NFREE], mybir.dt.bfloat16)
            target = pool.tile([128, B_PER_CORE, WIDTH], mybir.dt.uint8)
            context = pool.tile([128, B_PER_CORE, WIDTH], mybir.dt.uint8)

            # chunk c covers j' in [1024c, 1024c+1024) of both halves
            for c in range(2):
                for h in range(2):
                    cs = slice(JW * h + (JW // 2) * c,
                               JW * h + (JW // 2) * (c + 1))
                    nc.vector.tensor_scalar(
                        out=rhs[:, cs], in0=Jc[c],
                        scalar1=g[:, 4 + h: 5 + h], scalar2=None,
                        op0=A.is_ge)
                for h in range(2):
                    # 2-bank psum tile: two matmuls fill it, one wide ACT
                    # copy drains it (amortizes the PSUM access latency)
                    ps = psum_pool.tile([128, 8, 128], mybir.dt.float32)
                    for k, gg in enumerate((2 * c, 2 * c + 1)):
                        gi = 4 * h + gg
                        nc.tensor.matmul(
                            ps[:, 4 * k: 4 * k + 4, :], lhs[h],
                            rhs[:, 512 * gi: 512 * (gi + 1)],
                            start=True, stop=True)
                    b2 = slice(16 * h + 8 * c, 16 * h + 8 * (c + 1))
                    # raw counts; host maps nonzero->True
                    nc.scalar.activation(
                        out=target[:, b2, :], in_=ps, func=F.Copy)
                    # context on DVE only - GpSimd's software ALU takes
                    # ~8us per [128,512] op on HW (measured). The final
                    # pair reads PSUM directly so it runs in parallel
                    # with its ACT copy instead of after it.
                    if h == 1 and c == 1:
                        nc.vector.tensor_scalar(
                            out=context[:, b2, :], in0=ps,
                            scalar1=0.0, scalar2=None, op0=A.is_le)
                    else:
                        nc.vector.tensor_scalar(
                            out=context[:, b2, :], in0=target[:, b2, :],
                            scalar1=0.0, scalar2=None, op0=A.is_equal)
                    if h == 1 and c == 0:
                        # batches 16-24 final as soon as their copy lands:
                        # issue their tmask DMA early so the tail's last
                        # DMA shrinks to 8 batches
                        nc.sync.dma_start(out=tmask[:, b2, :],
                                          in_=target[:, b2, :])
                if c == 1:
                    h0, h1b = slice(0, 16), slice(24, 32)
                    nc.sync.dma_start(out=tmask[:, h0, :],
                                      in_=target[:, h0, :])
                    nc.sync.dma_start(out=cmask[:, h0, :],
                                      in_=context[:, h0, :])
                    nc.sync.dma_start(out=cmask[:, 16:32, :],
                                      in_=context[:, 16:32, :])
                    nc.sync.dma_start(out=tmask[:, h1b, :],
                                      in_=target[:, h1b, :])
    if orig_dab is not None:
        tile.TileContext._drain_and_barrier = orig_dab
    nc.compile()
    return nc


def _get_graph():
    global _GRAPH
    if _GRAPH is None:
        _GRAPH = _build_graph()
    return _GRAPH


def _install_ntff_shim():
    """The agent image's antenv lacks axon_hooks; recreate it so
    run_bass_kernel_spmd(trace=True) can profile via the axon .so."""
    import sys
    import types
    if "antenv.axon_hooks" in sys.modules:
        return
    try:
        from trn_agent_boot.trn_boot import _ntff_profile_via_ctypes
        hook = _ntff_profile_via_ctypes("/opt/axon/libaxon_pjrt.so")
        mod = types.ModuleType("antenv.axon_hooks")
        mod._hook = hook
        mod.get_axon_ntff_profile_hook = lambda: mod._hook
        mod.set_axon_ntff_profile_hook = lambda h: setattr(mod, "_hook", h)
        sys.modules["antenv.axon_hooks"] = mod
        import antenv
        antenv.axon_hooks = mod
    except Exception as e:  # degrade to no profiling
        print(f"ntff shim install failed: {e}")


def kernel(**inputs):
    global last_exec_time_ns, last_results
    from concourse.bass_utils import run_bass_kernel_spmd

    tops, bhs, lefts, bws = _host_geometry(
        inputs["scales_u"], inputs["rand_top"], inputs["rand_left"],
        inputs.get("height", HEIGHT), inputs.get("width", WIDTH))

    jconst_arr = np.arange(JW, dtype=np.int16).reshape(1, JW)
    in_maps = []
    b_local = (np.arange(P_PER_CORE, dtype=np.int32) // NUM_TARGET_BLOCKS)
    half = np.arange(P_PER_CORE) >= 64        # partition >= 64 -> negative arm
    for d in range(N_CORES):
        sl = slice(P_PER_CORE * d, P_PER_CORE * (d + 1))
        t, h, l, w = tops[sl], bhs[sl], lefts[sl], bws[sl]
        bot = t + h
        L = b_local * WIDTH + l
        # The backend's rounding cast can give left == max_lefts, i.e.
        # left + w = WIDTH + 1: the reference clips at the grid edge, so
        # clamp R to the row boundary (the flattened-j interval would
        # otherwise wrap into the next batch's column 0).
        R = np.minimum(L + w, (b_local + 1) * WIDTH)
        gm = np.zeros((P_PER_CORE, 8 + 256), dtype=np.float32)
        rr = np.arange(128)[None, :]
        for hh in range(2):
            # block handled by partition p for half hh: q = 64*hh + p%64
            q = 64 * hh + (np.arange(P_PER_CORE) % 64)
            AA = np.where(half, bot[q], t[q])[:, None]
            BB = np.where(half, t[q], bot[q])[:, None]
            gm[:, 8 + 128 * hh: 136 + 128 * hh] = (
                (rr >= AA).astype(np.float32) - (rr >= BB))
            gm[:, 4 + hh] = np.where(half, R[q], L[q]) - JW * hh
        in_maps.append({"geom": gm, "jconst": jconst_arr})

    nc = _get_graph()
    trace = bool(os.environ.get("BMG_TRACE"))
    if trace:
        _install_ntff_shim()
    br = run_bass_kernel_spmd(nc, in_maps, core_ids=list(range(N_CORES)),
                              trace=trace)
    last_exec_time_ns = br.exec_time_ns
    last_results = br

    tgt = np.empty((BATCH, SEQ_LEN), dtype=bool)
    ctx = np.empty((BATCH, SEQ_LEN), dtype=bool)
    for d in range(N_CORES):
        r = br.results[d]
        bs = slice(B_PER_CORE * d, B_PER_CORE * (d + 1))
        tgt[bs] = (np.ascontiguousarray(r["tmask"].transpose(1, 0, 2))
                   .reshape(B_PER_CORE, SEQ_LEN) != 0)
        ctx[bs] = (np.ascontiguousarray(r["cmask"].transpose(1, 0, 2))
                   .reshape(B_PER_CORE, SEQ_LEN) != 0)
    return ctx, tgt


# revision 59
# speedup vs baseline: 1.0641x; 1.0641x over previous
"""BlockMaskGenerator Trainium2 kernel.

Reference semantics: for each of 256 batch items, 4 random rectangular
blocks are placed on a 128x128 grid; target_mask = union of the blocks
(flattened to 16384), context_mask = complement.

Strategy:
  * Host: geometry via the same eager jax ops as the reference (the
    neuron backend's float->int cast rounds, numpy would truncate);
    1024 elements of scalar math - negligible.
  * Data-parallel over 8 NeuronCores: core d handles batches [32d, 32d+32)
    = blocks [128d, 128d+128).
  * Device per core. Output cell j = 128*b + c; interval indicator
    [L <= j < R] = [j >= L] - [j >= R] (difference of step functions).
    The output is computed in 2 halves of 16 batches = 64 blocks each,
    so the difference is absorbed into the K=128 matmul contraction:
    partitions 0-63 carry +row_q * [j >= L_q], partitions 64-127 carry
    -row_q * [j >= R_q] (sign flip encoded by swapping row thresholds
    host-side; all device ops are plain is_ge):
      - J[p, j'] = j' for j' in [0, 2048): Pool iota, 2 chunks in
        separate tiles (Tile deps are per-tile); both output halves
        reuse J with thresholds shifted by -2048
      - rhs[p, j'] = [j' >= colthr_h[p]]  (4 is_ge chunks on DVE, 4x)
      - lhsT_h[p, r] = [r >= A_h[p]] - [r >= B_h[p]]  (tiny, row iota
        rides in the geom input)
      - psum_g = lhsT_h.T @ rhs slice = # blocks covering cell (exact
        small ints >= 0)
      - target bytes = Copy(psum) -> uint8 counts on ScalarE (host maps
        nonzero -> True; no Sign act-table load)
      - context = (count == 0) on DVE (GpSimd ALU measured ~8us/op)
      - per-half output DMAs [128(r), 16(b), 128(c)] uint8; host
        transposes during unshard.
  * The framework's init all-engine barrier (~4us of cold drains) guards
    only unused const-tile memsets; it and the duplicated tail barriers
    are patched to cheaper forms at build time.
"""

import os

import numpy as np

NUM_TARGET_BLOCKS = 4
SCALE_MIN = 0.15
SCALE_MAX = 0.2
ASPECT_RATIO = 0.75
BATCH = 256
HEIGHT = 128
WIDTH = 128
SEQ_LEN = HEIGHT * WIDTH
N_CORES = 8
B_PER_CORE = BATCH // N_CORES                # 32
P_PER_CORE = B_PER_CORE * NUM_TARGET_BLOCKS  # 128 blocks per core
NFREE = B_PER_CORE * WIDTH                   # 4096
JW = NFREE // 2                              # 2048

_GRAPH = None
last_exec_time_ns = None
last_results = None


def _host_geometry(scales_u, rand_top, rand_left, height=HEIGHT, width=WIDTH):
    """Geometry computed with the SAME eager jax ops as the reference.

    The reference runs eagerly on this environment's jax backend (neuron),
    whose float->int cast rounds instead of truncating; replicating the
    exact op sequence on the same backend gives bit-identical geometry
    (verified: 0/4.2M mask mismatches, vs 1.3% for a numpy mirror).
    """
    import jax.numpy as jnp

    scales = SCALE_MIN + scales_u * (SCALE_MAX - SCALE_MIN)
    block_areas = (scales * height * width).astype(jnp.int32)
    block_hs = jnp.clip(
        jnp.sqrt(block_areas.astype(jnp.float32) / ASPECT_RATIO).astype(jnp.int32),
        1, height)
    block_ws = jnp.clip(
        (block_areas.astype(jnp.float32)
         / jnp.clip(block_hs, 1, None).astype(jnp.float32)).astype(jnp.int32),
        1, width)
    max_tops = jnp.clip(height - block_hs + 1, 1, None)
    max_lefts = jnp.clip(width - block_ws + 1, 1, None)
    tops = (rand_top * max_tops.astype(jnp.float32)).astype(jnp.int32)
    lefts = (rand_left * max_lefts.astype(jnp.float32)).astype(jnp.int32)
    return (np.asarray(tops), np.asarray(block_hs),
            np.asarray(lefts), np.asarray(block_ws))


def _build_graph():
    import concourse.bacc as bacc
    import concourse.bass as bass
    import concourse.tile as tile
    from concourse import mybir

    skip_init_barrier = os.environ.get("BMG_INIT_BARRIER", "skip") == "skip"
    if skip_init_barrier:
        # Bass.__init__'s all-engine barrier only guards const-ap memsets
        # (unused by this kernel: Copy keeps a float bias, compares use AP
        # scalars). Cold InstDrains cost ~3us per engine on HW.
        orig_barrier = bass.Bass.all_engine_barrier
        bass.Bass.all_engine_barrier = lambda self, **k: None

    try:
        # Bacc (not plain Bass): compile() runs generate_event_semaphores,
        # which splits multi-sem waits (TRN2 instructions have one
        # sync-wait slot; raw Tile output fails walrus codegen).
        nc = bacc.Bacc()
    finally:
        if skip_init_barrier:
            bass.Bass.all_engine_barrier = orig_barrier

    # geom columns: 0=rowA_h0 1=rowB_h0 2=rowA_h1 3=rowB_h1
    #               4=colthr_h0 5=colthr_h1 6=0 7=pad 8..135=row iota (r)
    geom = nc.declare_dram_parameter(
        "geom", [128, 8 + 128], mybir.dt.float32, isOutput=False)
    jconst = nc.declare_dram_parameter(
        "jconst", [1, JW], mybir.dt.int16, isOutput=False)
    tmask = nc.declare_dram_parameter(
        "tmask", [128, B_PER_CORE, WIDTH], mybir.dt.uint8, isOutput=True)
    cmask = nc.declare_dram_parameter(
        "cmask", [128, B_PER_CORE, WIDTH], mybir.dt.uint8, isOutput=True)

    A = mybir.AluOpType
    F = mybir.ActivationFunctionType

    if os.environ.get("BMG_TAIL_BARRIER", "sem") == "sem":
        # The tail's two all-engine barriers only order the semaphore
        # cleanup after the (kept) completion drain; sequencer-level
        # barriers suffice and skip another round of ~1us InstDrains.
        def _dab(self, tick_clock, wait_clock):
            drain_inst = self.nc.sync.drain()
            wait_clock.add_sem_waits(
                drain_inst.ins,
                tile.ScopedClock({None: tick_clock.global_clock}))
            hand = self.nc.alloc_semaphore("tail_handoff")
            drain_inst.then_inc(hand, 1)
            self.nc.gpsimd.wait_ge(hand, 1)
            popped = self.nc._tile_sem_poison_stack.pop()
            assert popped is self._sem_poison
            self.nc.clear_and_free_semaphores(
                list(self.sems.allocated().values()))
            self.nc.gpsimd.sem_clear(hand)
            # no trailing barrier: the clears are already the last
            # instructions on their engine queue; NRT waits for all
            # queues to drain at NEFF end regardless
        tc_cls = tile.TileContext
        orig_dab = tc_cls._drain_and_barrier
        tc_cls._drain_and_barrier = _dab
    else:
        orig_dab = None

    with tile.TileContext(nc) as tc:
        with (
            tc.tile_pool(name="sbuf", bufs=1) as pool,
            tc.tile_pool(name="tmp", bufs=2) as tmp,
            tc.tile_pool(name="psum", bufs=4, space="PSUM") as psum_pool,
        ):
            g = pool.tile([128, 8 + 128], mybir.dt.float32)
            nc.sync.dma_start(out=g, in_=geom[:, :])

            # J[p, j'] = j' in [0, 2048): partition-broadcast DMAs of a
            # host arange, 2 chunks in separate tiles (Tile deps are
            # per-tile). Pool iota is avoided - a running iota stalls
            # every concurrent DVE op (GpSimd/DVE SBUF port sharing).
            # Hybrid: chunk 0 via Pool iota (finishes before DVE has
            # inputs, so its port-conflict stall is free), chunk 1 via
            # partition-broadcast DMA (a second iota would stall DVE
            # right when work arrives).
            import concourse.bass as bass
            J0 = pool.tile([128, JW // 2], mybir.dt.int16, tag="J0")
            nc.gpsimd.iota(J0, pattern=[[1, JW // 2]], base=0,
                           channel_multiplier=0)
            jhalf = jconst.ap()[:, JW // 2:]
            J1 = pool.tile([128, JW // 2], mybir.dt.int16, tag="J1")
            nc.sync.dma_start(out=J1, in_=bass.AP(
                tensor=jhalf.tensor, offset=jhalf.offset,
                ap=[[0, 128]] + jhalf.ap[1:]))
            Jc = [J0, J1]

            # lhsT per half: [r >= A] - [r >= B] (sign flip via host swap)
            lhs = []
            for h in range(2):
                ra = tmp.tile([128, 128], mybir.dt.bfloat16, tag="ra")
                nc.vector.tensor_scalar(
                    out=ra, in0=g[:, 8:136], scalar1=g[:, 2 * h: 2 * h + 1],
                    scalar2=None, op0=A.is_ge)
                rb = tmp.tile([128, 128], mybir.dt.bfloat16, tag="rb")
                nc.vector.tensor_scalar(
                    out=rb, in0=g[:, 8:136],
                    scalar1=g[:, 2 * h + 1: 2 * h + 2],
                    scalar2=None, op0=A.is_ge)
                lh = pool.tile([128, 128], mybir.dt.bfloat16, tag=f"lh{h}")
                nc.vector.tensor_tensor(out=lh, in0=ra, in1=rb,
                                        op=A.subtract)
                lhs.append(lh)

            rhs = pool.tile([128, NFREE], mybir.dt.bfloat16)
            target = pool.tile([128, B_PER_CORE, WIDTH], mybir.dt.uint8)
            context = pool.tile([128, B_PER_CORE, WIDTH], mybir.dt.uint8)

            # chunk c covers j' in [1024c, 1024c+1024) of both halves
            for c in range(2):
                for h in range(2):
                    cs = slice(JW * h + (JW // 2) * c,
                               JW * h + (JW // 2) * (c + 1))
                    nc.vector.tensor_scalar(
                        out=rhs[:, cs], in0=Jc[c],
                        scalar1=g[:, 4 + h: 5 + h], scalar2=None,
                        op0=A.is_ge)
                for h in range(2):
                    # 2-bank psum tile: two matmuls fill it, one wide ACT
                    # copy drains it (amortizes the PSUM access latency)
                    ps = psum_pool.tile([128, 8, 128], mybir.dt.float32)
                    for k, gg in enumerate((2 * c, 2 * c + 1)):
                        gi = 4 * h + gg
                        nc.tensor.matmul(
                            ps[:, 4 * k: 4 * k + 4, :], lhs[h],
                            rhs[:, 512 * gi: 512 * (gi + 1)],
                            start=True, stop=True)
                    b2 = slice(16 * h + 8 * c, 16 * h + 8 * (c + 1))
                    # raw counts; host maps nonzero->True
                    nc.scalar.activation(
                        out=target[:, b2, :], in_=ps, func=F.Copy)
                    # context on DVE only - GpSimd's software ALU takes
                    # ~8us per [128,512] op on HW (measured). The final
                    # pair reads PSUM directly so it runs in parallel
                    # with its ACT copy instead of after it.
                    if h == 1 and c == 1:
                        nc.vector.tensor_scalar(
                            out=context[:, b2, :], in0=ps,
                            scalar1=0.0, scalar2=None, op0=A.is_le)
                    else:
                        nc.vector.tensor_scalar(
                            out=context[:, b2, :], in0=target[:, b2, :],
                            scalar1=0.0, scalar2=None, op0=A.is_equal)
                    if h == 1 and c == 0:
                        # batches 16-24 final as soon as their copy lands:
                        # issue their tmask DMA early so the tail's last
                        # DMA shrinks to 8 batches
                        nc.sync.dma_start(out=tmask[:, b2, :],
                                          in_=target[:, b2, :])
                if c == 1:
                    h0, h1b = slice(0, 16), slice(24, 32)
                    nc.sync.dma_start(out=tmask[:, h0, :],
                                      in_=target[:, h0, :])
                    nc.sync.dma_start(out=cmask[:, h0, :],
                                      in_=context[:, h0, :])
                    nc.sync.dma_start(out=cmask[:, 16:32, :],
                                      in_=context[:, 16:32, :])
                    nc.sync.dma_start(out=tmask[:, h1b, :],
                                      in_=target[:, h1b, :])
    if orig_dab is not None:
        tile.TileContext._drain_and_barrier = orig_dab
    nc.compile()
    return nc


def _get_graph():
    global _GRAPH
    if _GRAPH is None:
        _GRAPH = _build_graph()
    return _GRAPH


def _install_ntff_shim():
    """The agent image's antenv lacks axon_hooks; recreate it so
    run_bass_kernel_spmd(trace=True) can profile via the axon .so."""
    import sys
    import types
    if "antenv.axon_hooks" in sys.modules:
        return
    try:
        from trn_agent_boot.trn_boot import _ntff_profile_via_ctypes
        hook = _ntff_profile_via_ctypes("/opt/axon/libaxon_pjrt.so")
        mod = types.ModuleType("antenv.axon_hooks")
        mod._hook = hook
        mod.get_axon_ntff_profile_hook = lambda: mod._hook
        mod.set_axon_ntff_profile_hook = lambda h: setattr(mod, "_hook", h)
        sys.modules["antenv.axon_hooks"] = mod
        import antenv
        antenv.axon_hooks = mod
    except Exception as e:  # degrade to no profiling
        print(f"ntff shim install failed: {e}")


def kernel(**inputs):
    global last_exec_time_ns, last_results
    from concourse.bass_utils import run_bass_kernel_spmd

    tops, bhs, lefts, bws = _host_geometry(
        inputs["scales_u"], inputs["rand_top"], inputs["rand_left"],
        inputs.get("height", HEIGHT), inputs.get("width", WIDTH))

    jconst_arr = np.arange(JW, dtype=np.int16).reshape(1, JW)
    in_maps = []
    b_local = (np.arange(P_PER_CORE, dtype=np.int32) // NUM_TARGET_BLOCKS)
    half = np.arange(P_PER_CORE) >= 64        # partition >= 64 -> negative arm
    for d in range(N_CORES):
        sl = slice(P_PER_CORE * d, P_PER_CORE * (d + 1))
        t, h, l, w = tops[sl], bhs[sl], lefts[sl], bws[sl]
        bot = t + h
        L = b_local * WIDTH + l
        # The backend's rounding cast can give left == max_lefts, i.e.
        # left + w = WIDTH + 1: the reference clips at the grid edge, so
        # clamp R to the row boundary (the flattened-j interval would
        # otherwise wrap into the next batch's column 0).
        R = np.minimum(L + w, (b_local + 1) * WIDTH)
        gm = np.zeros((P_PER_CORE, 8 + 128), dtype=np.float32)
        gm[:, 8:] = np.arange(128, dtype=np.float32)[None, :]
        for hh in range(2):
            # block handled by partition p for half hh: q = 64*hh + p%64
            q = 64 * hh + (np.arange(P_PER_CORE) % 64)
            gm[:, 2 * hh] = np.where(half, bot[q], t[q])
            gm[:, 2 * hh + 1] = np.where(half, t[q], bot[q])
            gm[:, 4 + hh] = np.where(half, R[q], L[q]) - JW * hh
        in_maps.append({"geom": gm, "jconst": jconst_arr})

    nc = _get_graph()
    trace = bool(os.environ.get("BMG_TRACE"))
    if trace:
        _install_ntff_shim()
    br = run_bass_kernel_spmd(nc, in_maps, core_ids=list(range(N_CORES)),
                              trace=trace)
    last_exec_time_ns = br.exec_time_ns
    last_results = br

    tgt = np.empty((BATCH, SEQ_LEN), dtype=bool)
    ctx = np.empty((BATCH, SEQ_LEN), dtype=bool)
    for d in range(N_CORES):
        r = br.results[d]
        bs = slice(B_PER_CORE * d, B_PER_CORE * (d + 1))
        tgt[bs] = (np.ascontiguousarray(r["tmask"].transpose(1, 0, 2))
                   .reshape(B_PER_CORE, SEQ_LEN) != 0)
        ctx[bs] = (np.ascontiguousarray(r["cmask"].transpose(1, 0, 2))
                   .reshape(B_PER_CORE, SEQ_LEN) != 0)
    return ctx, tgt
